# revision 18
# baseline (speedup 1.0000x reference)
"""Bass/Trainium2 kernel for nn_DynamicEdgeWeights.

Math (B=4, S=512, D=128, H=8):
    a = x @ w1[:D]; c = x @ w1[D:]
    h[b,i,j,:] = relu(a[b,i,:] + c[b,j,:] + b1)
    out[b,h,i,j] = sigmoid(sum_d h[b,i,j,d] * w2[d,h] + b2[h])

Device strategy (per core; 8 cores, core k -> batch k//2, i-rows [(k%2)*256, +256)):
  - cT[d, j] = (x[b] @ w1c).T and aT[d, i] = (x[b] @ w1a).T + b1 via two PE
    matmuls on pre-transposed x (host passes x[b].T).
  - per query row i: one fused relu(cT + aT[:, i]) producing h_i [128d, 512j]
    (DVE tensor_scalar add+max, or ACT activation Relu with per-partition bias).
  - second matmul uses "comb" weights: 16 query rows share one PSUM bank.
    comb_g [128, 128] has w2[:, h] in column h*16+g, zeros elsewhere; 16
    accumulating matmuls put e-pre for (16 i x 8 h) on 128 PSUM partitions.
  - groups are processed in pairs sharing a 2-bank PSUM tile; one full-width
    [128, 1024] sigmoid (ACT, bias=b2 broadcast) -> SBUF -> stores split
    across both HWDGE queues straight into out[b, :, i-rows, :].
"""

import os
import sys

for _p in ("/opt/trn_rl_repo", "/root/.axon_site/_ro/trn_rl_repo"):
    if os.path.isdir(_p) and _p not in sys.path:
        sys.path.insert(0, _p)
        break

import numpy as np
import ml_dtypes  # noqa: F401  (registers bfloat16 dtype)

import concourse.bass as bass  # noqa: F401  (registers types)
import concourse.mybir as mybir
from concourse import bacc
from concourse.tile import TileContext

B, S, D, H = 4, 512, 128, 8
N_CORES = 8
I_PER_CORE = (B * S) // N_CORES  # 256
G = 16  # query rows packed per PSUM bank
T = I_PER_CORE // G  # 16 groups per core
# ACT h-gen share: values >= 8 mean (n_act - 8) of the last 8 sched entries
# per pair go to ScalarE (rest DVE); 13 -> 5 of 32 rows per pair on ACT.
# Values 100+k mean k rows per pair spread EVENLY through the 32 sched
# entries (measured balance: DVE 303.5ns/row vs ACT 632ns/row + 1213ns
# sigmoid per pair -> optimum ~9/pair).
N_ACT = 109

F32 = mybir.dt.float32
F16 = mybir.dt.float16  # h-path dtype: full PE rate (fp32 streams at 1/4 rate)
F8 = mybir.dt.float8e4  # optional ACT-row dtype (ACT writes 1-byte faster)
F16_NP = "float16"

_CACHE: dict = {}


def _build_nc(loop_iters: int = 1, dt_h=F16, n_act=N_ACT, h_bufs=8, o_bufs=8, mm_bufs=4, staggered=False, diag=None, store_eng="one2h", mm_order="rqu", act_f8=False, store_f16=False, c_bufs=2, pipe=True, prep_eng="act", a_bufs=3):
    """Build the single-core Bass program (identical across the 8 cores).

    loop_iters > 1 wraps the whole compute in an on-device For_i repeat —
    used only for steady-state timing (one dispatch, N executions).
    """
    nc = bacc.Bacc(
        "TRN2",
        target_bir_lowering=False,
        debug=False,
        enable_asserts=False,
        num_devices=N_CORES,
    )

    # single packed constants tensor: [xj | w1c | xi | w1a | b1 | b2v | comb]
    # (comb fp16 bit-packed into f32 words) -> ONE head DMA instead of 7
    # serialized ~630ns DGE issues; the For_i reset barrier makes the head
    # serial, so issue count is on the critical path.
    PACK_W = S + D + I_PER_CORE + D + 1 + 1 + 64
    pack_d = nc.dram_tensor("pack", (D, PACK_W), F32, kind="ExternalInput").ap()
    dt_o = F16 if store_f16 else F32
    if store_eng.startswith("one2"):
        # raw drain-major layout: [drain, (g,h) partition, (u,j)]; host
        # un-permutes in _gather. Stores are fully contiguous.
        out_d = nc.dram_tensor(
            "out", (T // 2, D, 2 * S), dt_o, kind="ExternalOutput"
        ).ap()
    else:
        out_d = nc.dram_tensor(
            "out", (H, I_PER_CORE, S), dt_o, kind="ExternalOutput"
        ).ap()

    relu = mybir.ActivationFunctionType.Relu
    sigmoid = mybir.ActivationFunctionType.Sigmoid
    add = mybir.AluOpType.add
    amax = mybir.AluOpType.max

    import contextlib

    with TileContext(nc) as tc:
        with (
            # bufs=2: next iteration's loads + cT/aT precompute overlap this
            # iteration's tail instead of WAR-blocking on the single buffer
            tc.tile_pool(name="const", bufs=c_bufs) as cpool,
            tc.tile_pool(name="h", bufs=h_bufs) as hpool,
            tc.tile_pool(name="o", bufs=o_bufs) as opool,
            tc.tile_pool(name="mm", bufs=mm_bufs, space="PSUM") as mmpool,
            (
                tc.For_i(
                    0,
                    loop_iters,
                    1,
                    hint_engines=(
                        mybir.EngineType.PE,
                        mybir.EngineType.DVE,
                        mybir.EngineType.Activation,
                        mybir.EngineType.SP,
                    ),
                    staggered_reset=staggered,
                )
                if loop_iters > 1
                else contextlib.nullcontext()
            ),
        ):
            pack_sb = cpool.tile([D, PACK_W], F32)
            # two DMAs: the [xj|w1c] prefix gates the cT matmul -> its
            # completion must not wait for the rest of the pack
            P1 = S + D
            nc.sync.dma_start(out=pack_sb[:, :P1], in_=pack_d[:, :P1])
            nc.sync.dma_start(out=pack_sb[:, P1:], in_=pack_d[:, P1:])
            o0 = 0
            xj_sb = pack_sb[:, o0 : o0 + S]; o0 += S
            w1c_sb = pack_sb[:, o0 : o0 + D]; o0 += D
            xi_sb = pack_sb[:, o0 : o0 + I_PER_CORE]; o0 += I_PER_CORE
            w1a_sb = pack_sb[:, o0 : o0 + D]; o0 += D
            b1_sb = pack_sb[:, o0 : o0 + 1]; o0 += 1
            b2v_sb = pack_sb[:, o0 : o0 + 1]; o0 += 1
            comb_sb = pack_sb[:, o0 : o0 + 64].bitcast(dt_h)

            # precompute borrows one pair-slot from the matmul psum pool:
            # cT in the first bank-half, aT in the second
            pre_ps = mmpool.tile([D, 2 * S], F32, tag="ps2")
            # cT[d_out, j] = sum_k w1c[k, d_out] * xT[k, j]
            nc.tensor.matmul(pre_ps[:, :S], w1c_sb, xj_sb, start=True, stop=True)
            cT_sb = cpool.tile([D, S], dt_h)
            # aT[d_out, i] = sum_k w1a[k, d_out] * xT[k, i]  (+ b1 per partition)
            nc.tensor.matmul(
                pre_ps[:, S : S + I_PER_CORE], w1a_sb, xi_sb, start=True, stop=True
            )
            at_sb = cpool.tile([D, I_PER_CORE], F32)
            if prep_eng == "dve":
                nc.vector.tensor_copy(cT_sb, pre_ps[:, :S])
                nc.vector.tensor_scalar(
                    at_sb,
                    pre_ps[:, S : S + I_PER_CORE],
                    b1_sb,
                    0.0,
                    mybir.AluOpType.add,
                    mybir.AluOpType.bypass,
                )
            else:
                # on ACT (both funcs are in the resident sigmoid_and_others
                # table set); b1 is folded into cT (z = (c+b1) + a), so aT
                # is a plain copy with no bias dependency
                nc.scalar.activation(
                    cT_sb,
                    pre_ps[:, :S],
                    mybir.ActivationFunctionType.Identity,
                    bias=b1_sb,
                )
                nc.scalar.activation(
                    at_sb,
                    pre_ps[:, S : S + I_PER_CORE],
                    mybir.ActivationFunctionType.Copy,
                )

            def drain(t, ps2):
                # sigmoid + store for a finished pair of groups (t, t+1);
                # emitted one pair late so ACT's (stalling) sigmoid sits
                # behind the next pair's h-gen ops in ACT program order.
                o_sb = opool.tile([D, 2 * S], dt_o)
                nc.scalar.activation(o_sb, ps2, sigmoid, bias=b2v_sb)
                if store_eng.startswith("one2"):
                    # single fully-contiguous 256KB store per drain
                    if store_eng == "one2a":
                        eng = nc.scalar
                    elif store_eng == "one2s":
                        eng = nc.sync
                    elif store_eng == "one2h":
                        # early drains on SP (its loads are done by then and
                        # finish long before next iteration's loads); late
                        # drains on ACT so iteration-end stores never block
                        # the next iteration's SP load queue
                        eng = nc.sync if (t // 2) < 4 else nc.scalar
                    else:  # one2: alternate queues per drain
                        eng = nc.sync if (t // 2) % 2 == 0 else nc.scalar
                    eng.dma_start(out=out_d[t // 2], in_=o_sb)
                    return
                # partition p = g*8+h  ->  out[h, (t+u)*16+g, :]
                half = D // 2
                for u in range(2):
                    dst = out_d[:, (t + u) * G : (t + u + 1) * G, :].rearrange(
                        "h g j -> g h j"
                    )
                    src = o_sb[:, u * S : (u + 1) * S]
                    if store_eng == "big":
                        # one full-width store per u-half; alternate engines
                        eng = nc.sync if u == 0 else nc.scalar
                        eng.dma_start(out=dst, in_=src)
                    elif store_eng == "bigsp":
                        nc.sync.dma_start(out=dst, in_=src)
                    else:  # "split": halves across SP + ACT queues
                        nc.sync.dma_start(out=dst[: G // 2], in_=src[:half])
                        nc.scalar.dma_start(out=dst[G // 2 :], in_=src[half:])

            if diag == "pe":
                # PE-pure stream: one static h tile, full matmul schedule
                h_static = cpool.tile([D, 2 * S], dt_h)
                nc.vector.tensor_copy(h_static[:, :S], cT_sb)
                nc.vector.tensor_copy(h_static[:, S:], cT_sb)
                for t in range(0, T, 2):
                    ps2 = mmpool.tile([D, 2 * S], F32, tag="ps2")
                    g_order = [4 * q + r for r in range(4) for q in range(4)]
                    for n, g in enumerate(g_order):
                        q, r = g // 4, g % 4
                        for u in range(2):
                            nc.tensor.matmul(
                                ps2[32 * q : 32 * (q + 1), u * S : (u + 1) * S],
                                comb_sb[:, 32 * r : 32 * (r + 1)],
                                h_static[:, u * S : (u + 1) * S],
                                start=(r == 0),
                                stop=(r == 3),
                                tile_position=(0, 32 * q),
                                skip_group_check=True,
                            )
                    o_sb = opool.tile([D, 2 * S], F32)
                    nc.scalar.activation(o_sb, ps2, sigmoid, bias=b2v_sb)
                    half = D // 2
                    for u in range(2):
                        dst = out_d[:, (t + u) * G : (t + u + 1) * G, :]
                        src = o_sb[:, u * S : (u + 1) * S]
                        nc.sync.dma_start(out=dst[: H // 2], in_=src[:half])
                        nc.scalar.dma_start(out=dst[H // 2 :], in_=src[half:])
            elif diag == "dve":
                # DVE-pure stream: all h-gen ops, no matmul/sigmoid; dump one
                # h tile to out to keep outputs written
                for t in range(0, T, 2):
                    for g in range(G):
                        h2 = hpool.tile([D, 2 * S], dt_h)
                        for u in range(2):
                            i_loc = (t + u) * G + g
                            a_col = at_sb[:, i_loc : i_loc + 1]
                            dst = h2[:, u * S : (u + 1) * S]
                            nc.vector.tensor_scalar(dst, cT_sb, a_col, 0.0, add, amax)
                    o_sb = opool.tile([D, 2 * S], F32)
                    nc.vector.tensor_copy(o_sb, h2)
                    half = D // 2
                    for u in range(2):
                        dst = out_d[:, (t + u) * G : (t + u + 1) * G, :]
                        src = o_sb[:, u * S : (u + 1) * S]
                        nc.sync.dma_start(out=dst[: H // 2], in_=src[:half])
                        nc.scalar.dma_start(out=dst[H // 2 :], in_=src[half:])
            elif not pipe:
                pending = None  # (t, psum tile) awaiting sigmoid+store
                # emission order: r outer, u middle, q fastest -> consecutive
                # matmuls land in 4 different 32-col PE strips (concurrent
                # streaming); a strip's accumulation chain (same q,u across r)
                # recurs only every 8 instructions.
                if mm_order == "ruq":
                    sched = [
                        (4 * q + r, u)
                        for r in range(4)
                        for u in range(2)
                        for q in range(4)
                    ]
                else:  # "rqu": the original order, u innermost
                    sched = [
                        (4 * q + r, u)
                        for r in range(4)
                        for q in range(4)
                        for u in range(2)
                    ]
                for t in range(0, T, 2):
                    # two groups (t, t+1) share one 2-bank PSUM tile: matmul g
                    # covers j 0..511 for row t*16+g and j 512..1023 for row
                    # (t+1)*16+g with the same comb_g weights.
                    ps2 = mmpool.tile([D, 2 * S], F32, tag="ps2")
                    if n_act >= 100:
                        # spread k ACT rows evenly over the 32 entries
                        act_k = n_act - 100
                        act_pos = {(i * 32) // act_k for i in range(act_k)}
                    else:
                        act_pos = None
                    for n, (g, u) in enumerate(sched):
                        q, r = g // 4, g % 4
                        i_loc = (t + u) * G + g
                        a_col = at_sb[:, i_loc : i_loc + 1]
                        if act_pos is not None:
                            on_act = n in act_pos
                        else:
                            # last act_k of the 32 sched entries go to ACT (all
                            # r==3 tail positions when act_k <= 8)
                            act_k = n_act * 2 if n_act < 8 else n_act - 8
                            on_act = n >= 32 - act_k
                        dt_row = F8 if (on_act and act_f8) else dt_h
                        hu = hpool.tile([D, S], dt_row, tag=f"h{u}{'f8' if dt_row is F8 else ''}")
                        if on_act:
                            nc.scalar.activation(hu, cT_sb, relu, bias=a_col)
                        else:
                            nc.vector.tensor_scalar(hu, cT_sb, a_col, 0.0, add, amax)
                        nc.tensor.matmul(
                            ps2[32 * q : 32 * (q + 1), u * S : (u + 1) * S],
                            comb_sb[:, 32 * r : 32 * (r + 1)],
                            hu,
                            start=(r == 0),
                            stop=(r == 3),
                            tile_position=(0, 32 * q),
                            skip_group_check=True,
                        )
                        if n == 3 and pending is not None:
                            drain(*pending)
                            pending = None
                    pending = (t, ps2)
                drain(*pending)
            else:
                # ACT-lookahead pipeline: ACT produces its h rows one pair
                # AHEAD of consumption so its (jittery, sigmoid-interleaved)
                # stream never stalls the PE's in-order consumption; only the
                # DVE remains tightly coupled.
                pending = None
                if mm_order == "ruq":
                    sched = [
                        (4 * q + r, u)
                        for r in range(4)
                        for u in range(2)
                        for q in range(4)
                    ]
                else:
                    sched = [
                        (4 * q + r, u)
                        for r in range(4)
                        for q in range(4)
                        for u in range(2)
                    ]
                act_k = (n_act - 100) if n_act >= 100 else max(0, n_act - 8)
                # offset by 2 so the PE's first consumption (and the last
                # pair's tail) is never ACT-gated
                act_pos = (
                    sorted({min(2 + (i * 30) // act_k, 31) for i in range(act_k)})
                    if act_k
                    else []
                )
                act_set = set(act_pos)
                act_tiles = {}
                # last pair runs u-major so its u=0 PSUM half finishes at
                # entry 15 and drains while the u=1 matmuls still run
                sched_last = [
                    (4 * q + r, u) for u in range(2) for r in range(4) for q in range(4)
                ]

                def sched_for(t):
                    return sched_last if t == T - 2 else sched

                def act_produce(t):
                    sch = sched_for(t)
                    for idx, n in enumerate(act_pos):
                        g, u = sch[n]
                        i_loc = (t + u) * G + g
                        ha = hpool.tile(
                            [D, S], dt_h, tag=f"A{idx}", bufs=a_bufs
                        )
                        nc.scalar.activation(
                            ha, cT_sb, relu, bias=at_sb[:, i_loc : i_loc + 1]
                        )
                        act_tiles[(t, n)] = ha

                def drain_half(t, u, ps2):
                    oh = opool.tile([D, S], dt_o, tag="oh")
                    nc.scalar.activation(
                        oh, ps2[:, u * S : (u + 1) * S], sigmoid, bias=b2v_sb
                    )
                    nc.scalar.dma_start(
                        out=out_d[t // 2, :, u * S : (u + 1) * S], in_=oh
                    )

                act_produce(0)
                for t in range(0, T, 2):
                    ps2 = mmpool.tile([D, 2 * S], F32, tag="ps2")
                    for n, (g, u) in enumerate(sched_for(t)):
                        q, r = g // 4, g % 4
                        i_loc = (t + u) * G + g
                        if n in act_set:
                            hu = act_tiles.pop((t, n))
                        else:
                            hu = hpool.tile([D, S], dt_h, tag=f"h{u}")
                            nc.vector.tensor_scalar(
                                hu,
                                cT_sb,
                                at_sb[:, i_loc : i_loc + 1],
                                0.0,
                                add,
                                amax,
                            )
                        nc.tensor.matmul(
                            ps2[32 * q : 32 * (q + 1), u * S : (u + 1) * S],
                            comb_sb[:, 32 * r : 32 * (r + 1)],
                            hu,
                            start=(r == 0),
                            stop=(r == 3),
                            tile_position=(0, 32 * q),
                            skip_group_check=True,
                        )
                        if n == 3 and pending is not None:
                            drain(*pending)
                            pending = None
                        if n == 5 and t + 2 < T:
                            act_produce(t + 2)
                        if t == T - 2 and n == 15:
                            drain_half(t, 0, ps2)
                    pending = (t, ps2)
                drain_half(T - 2, 1, pending[1])

    nc.compile()
    # Activation-table cleanup: the table pass puts LoadActFuncSet(0) (relu
    # set) AND LoadActFuncSet(2) (sigmoid_and_others, which also contains
    # Relu) inside the loop body — 2.6us of ACT per iteration. Set 2 serves
    # every activation here (verified bit-identical), so drop the set-0
    # loads and hoist the set-2 load into the preceding block so it runs
    # once instead of per iteration.
    blocks = nc.m.functions[0].blocks
    for bi, b in enumerate(blocks):
        b.instructions[:] = [
            i
            for i in b.instructions
            if not (isinstance(i, mybir.InstLoadActFuncSet) and i.act_func_set_id == 0)
        ]
        if bi > 0:
            hoist = [
                i for i in b.instructions if isinstance(i, mybir.InstLoadActFuncSet)
            ]
            if hoist:
                b.instructions[:] = [
                    i
                    for i in b.instructions
                    if not isinstance(i, mybir.InstLoadActFuncSet)
                ]
                for i in reversed(hoist):
                    blocks[bi - 1].instructions.insert(0, i)
    return nc


def _host_prep(node_features, w1, b1, w2, b2):
    """Shared (per-core-replicated) small tensors + per-core input maps."""
    w1a = np.ascontiguousarray(w1[:D])  # [D, D] == lhsT for aT
    w1c = np.ascontiguousarray(w1[D:])  # [D, D] == lhsT for cT
    b1c = np.ascontiguousarray(b1.reshape(D, 1))
    # psum partition p = g*8 + h; col-group q = g//4 covers partitions
    # [32q, 32q+32); weight tile r = g%4 has w2 in columns [8r, 8r+8)
    comb = np.zeros((D, 4, 32), np.float32)
    for r in range(4):
        comb[:, r, r * H : (r + 1) * H] = w2
    comb = np.ascontiguousarray(comb.reshape(D, 4 * 32).astype(F16_NP))
    b2v = np.ascontiguousarray(np.tile(b2, G).reshape(D, 1))

    comb_as_f32 = comb.view(np.float32)  # [D, 64]
    in_maps = []
    for k in range(N_CORES):
        b = k // (N_CORES // B)
        i0 = (k % (N_CORES // B)) * I_PER_CORE
        xT = np.ascontiguousarray(node_features[b].T)  # [D, S]
        pack = np.concatenate(
            [
                xT,
                w1c,
                xT[:, i0 : i0 + I_PER_CORE],
                w1a,
                b1c,
                b2v,
                comb_as_f32,
            ],
            axis=1,
        )
        in_maps.append({"pack": np.ascontiguousarray(pack)})
    return in_maps


def _gather(results):
    out = np.empty((B, H, S, S), np.float32)
    for k in range(N_CORES):
        b = k // (N_CORES // B)
        i0 = (k % (N_CORES // B)) * I_PER_CORE
        arr = results[k]["out"]
        if arr.dtype != np.float32:
            arr = arr.astype(np.float32)
        if arr.shape[0] == T // 2:  # raw drain-major layout (one2 stores)
            # arr[d, g*8+h, u*512+j] -> out[b, h, i0 + d*32+u*16+g, j]
            a5 = arr.reshape(T // 2, G, H, 2, S)  # [d, g, h, u, j]
            out[b, :, i0 : i0 + I_PER_CORE, :] = (
                a5.transpose(2, 0, 3, 1, 4).reshape(H, I_PER_CORE, S)
            )
        else:
            out[b, :, i0 : i0 + I_PER_CORE, :] = arr
    return out


def _build_jit(nc):
    """Single cached jit around the bass_exec custom call (the stock
    run_bass_kernel_spmd path re-traces/jits on every invocation)."""
    import jax
    from jax.sharding import Mesh, PartitionSpec

    try:
        from jax.experimental.shard_map import shard_map
    except ImportError:
        from jax.sharding import shard_map

    from concourse.bass2jax import (
        _bass_exec_p,
        install_neuronx_cc_hook,
        partition_id_tensor,
    )

    install_neuronx_cc_hook()
    partition_name = nc.partition_id_tensor.name if nc.partition_id_tensor else None
    in_names, out_names, out_avals, zero_outs = [], [], [], []
    for alloc in nc.m.functions[0].allocations:
        if not isinstance(alloc, mybir.MemoryLocationSet):
            continue
        name = alloc.memorylocations[0].name
        if alloc.kind == "ExternalInput":
            if name != partition_name:
                in_names.append(name)
        elif alloc.kind == "ExternalOutput":
            shape = tuple(alloc.tensor_shape)
            np_dt = mybir.dt.np(alloc.dtype)
            out_avals.append(jax.core.ShapedArray(shape, np_dt))
            out_names.append(name)
            zero_outs.append(np.zeros(shape, np_dt))
    n_params = len(in_names)
    all_in_names = list(in_names) + list(out_names)
    if partition_name is not None:
        all_in_names.append(partition_name)

    def _body(*args):
        operands = list(args)
        if partition_name is not None:
            operands.append(partition_id_tensor())
        return tuple(
            _bass_exec_p.bind(
                *operands,
                out_avals=tuple(out_avals),
                in_names=tuple(all_in_names),
                out_names=tuple(out_names),
                lowering_input_output_aliases=(),
                sim_require_finite=True,
                sim_require_nnan=True,
                nc=nc,
            )
        )

    devices = jax.devices()[:N_CORES]
    mesh = Mesh(np.asarray(devices), ("core",))
    n_outs = len(out_names)
    sharded = jax.jit(
        shard_map(
            _body,
            mesh=mesh,
            in_specs=(PartitionSpec("core"),) * (n_params + n_outs),
            out_specs=(PartitionSpec("core"),) * n_outs,
            check_rep=False,
        ),
        # no donation: the kernel writes every output element, so the zero
        # operand buffers can live on device and be reused across calls
        keep_unused=True,
    )
    return sharded, in_names, out_names, zero_outs


def _run(in_maps):
    if "nc" not in _CACHE:
        _CACHE["nc"] = _build_nc()
        _CACHE["jit"] = _build_jit(_CACHE["nc"])
    sharded, in_names, out_names, zero_outs = _CACHE["jit"]
    concat_in = [
        np.concatenate([np.asarray(in_maps[c][n]) for c in range(N_CORES)], axis=0)
        for n in in_names
    ]
    if "zeros_dev" not in _CACHE:
        import jax

        _CACHE["zeros_dev"] = [
            jax.device_put(np.zeros((N_CORES * z.shape[0], *z.shape[1:]), z.dtype))
            for z in zero_outs
        ]
    out_arrs = sharded(*concat_in, *_CACHE["zeros_dev"])
    # outputs come back concatenated on axis 0 (N_CORES * dim0, ...)
    split = []
    for i, name in enumerate(out_names):
        arr = np.asarray(out_arrs[i])
        split.append(arr.reshape(N_CORES, arr.shape[0] // N_CORES, *arr.shape[1:]))
    return [
        {name: split[i][c] for i, name in enumerate(out_names)}
        for c in range(N_CORES)
    ]


def kernel(node_features, w1, b1, w2, b2):
    node_features = np.asarray(node_features, np.float32)
    w1 = np.asarray(w1, np.float32)
    b1 = np.asarray(b1, np.float32)
    w2 = np.asarray(w2, np.float32)
    b2 = np.asarray(b2, np.float32)
    in_maps = _host_prep(node_features, w1, b1, w2, b2)
    results = _run(in_maps)
    return _gather(results)



# revision 23
# speedup vs baseline: 1.0643x; 1.0643x over previous
"""Bass/Trainium2 kernel for nn_DynamicEdgeWeights.

Math (B=4, S=512, D=128, H=8):
    a = x @ w1[:D]; c = x @ w1[D:]
    h[b,i,j,:] = relu(a[b,i,:] + c[b,j,:] + b1)
    out[b,h,i,j] = sigmoid(sum_d h[b,i,j,d] * w2[d,h] + b2[h])

Device strategy (per core; 8 cores, core k -> batch k//2, i-rows [(k%2)*256, +256)):
  - cT[d, j] = (x[b] @ w1c).T and aT[d, i] = (x[b] @ w1a).T + b1 via two PE
    matmuls on pre-transposed x (host passes x[b].T).
  - per query row i: one fused relu(cT + aT[:, i]) producing h_i [128d, 512j]
    (DVE tensor_scalar add+max, or ACT activation Relu with per-partition bias).
  - second matmul uses "comb" weights: 16 query rows share one PSUM bank.
    comb_g [128, 128] has w2[:, h] in column h*16+g, zeros elsewhere; 16
    accumulating matmuls put e-pre for (16 i x 8 h) on 128 PSUM partitions.
  - groups are processed in pairs sharing a 2-bank PSUM tile; one full-width
    [128, 1024] sigmoid (ACT, bias=b2 broadcast) -> SBUF -> stores split
    across both HWDGE queues straight into out[b, :, i-rows, :].
"""

import os
import sys

for _p in ("/opt/trn_rl_repo", "/root/.axon_site/_ro/trn_rl_repo"):
    if os.path.isdir(_p) and _p not in sys.path:
        sys.path.insert(0, _p)
        break

import numpy as np
import ml_dtypes  # noqa: F401  (registers bfloat16 dtype)

import concourse.bass as bass  # noqa: F401  (registers types)
import concourse.mybir as mybir
from concourse import bacc
from concourse.tile import TileContext

B, S, D, H = 4, 512, 128, 8
N_CORES = 8
I_PER_CORE = (B * S) // N_CORES  # 256
G = 16  # query rows packed per PSUM bank
T = I_PER_CORE // G  # 16 groups per core
# ACT h-gen share: values >= 8 mean (n_act - 8) of the last 8 sched entries
# per pair go to ScalarE (rest DVE); 13 -> 5 of 32 rows per pair on ACT.
# Values 100+k mean k rows per pair spread EVENLY through the 32 sched
# entries (measured balance: DVE 303.5ns/row vs ACT 632ns/row + 1213ns
# sigmoid per pair -> optimum ~9/pair).
N_ACT = 109

F32 = mybir.dt.float32
F16 = mybir.dt.float16  # h-path dtype: full PE rate (fp32 streams at 1/4 rate)
F8 = mybir.dt.float8e4  # optional ACT-row dtype (ACT writes 1-byte faster)
F16_NP = "float16"

_CACHE: dict = {}


def _build_nc(loop_iters: int = 1, dt_h=F16, n_act=N_ACT, h_bufs=8, o_bufs=8, mm_bufs=4, staggered=False, diag=None, store_eng="one2h", mm_order="rqu", act_f8=False, store_f16=True, c_bufs=2, pipe=True, prep_eng="act", a_bufs=3):
    """Build the single-core Bass program (identical across the 8 cores).

    loop_iters > 1 wraps the whole compute in an on-device For_i repeat —
    used only for steady-state timing (one dispatch, N executions).
    """
    nc = bacc.Bacc(
        "TRN2",
        target_bir_lowering=False,
        debug=False,
        enable_asserts=False,
        num_devices=N_CORES,
    )

    # single packed constants tensor, mostly fp16:
    # [xj | w1c | xi | w1a | comb]:f16  ++  [b1 | b2v]:f32 (bit-packed into
    # one f32 row-major buffer) -> ONE head DMA instead of 7 serialized
    # ~630ns DGE issues; the For_i reset barrier makes the head serial, so
    # both issue count and transfer size are on the critical path.
    PACK_H = S + D + I_PER_CORE + D + 4 * 32  # fp16 columns
    PACK_W = PACK_H // 2 + 2
    pack_d = nc.dram_tensor("pack", (D, PACK_W), F32, kind="ExternalInput").ap()
    dt_o = F16 if store_f16 else F32
    if store_eng.startswith("one2"):
        # raw drain-major layout: [drain, (g,h) partition, (u,j)]; host
        # un-permutes in _gather. Stores are fully contiguous.
        out_d = nc.dram_tensor(
            "out", (T // 2, D, 2 * S), dt_o, kind="ExternalOutput"
        ).ap()
    else:
        out_d = nc.dram_tensor(
            "out", (H, I_PER_CORE, S), dt_o, kind="ExternalOutput"
        ).ap()

    relu = mybir.ActivationFunctionType.Relu
    sigmoid = mybir.ActivationFunctionType.Sigmoid
    add = mybir.AluOpType.add
    amax = mybir.AluOpType.max

    import contextlib

    with TileContext(nc) as tc:
        with (
            # bufs=2: next iteration's loads + cT/aT precompute overlap this
            # iteration's tail instead of WAR-blocking on the single buffer
            tc.tile_pool(name="const", bufs=c_bufs) as cpool,
            tc.tile_pool(name="h", bufs=h_bufs) as hpool,
            tc.tile_pool(name="o", bufs=o_bufs) as opool,
            tc.tile_pool(name="mm", bufs=mm_bufs, space="PSUM") as mmpool,
            (
                tc.For_i(
                    0,
                    loop_iters,
                    1,
                    hint_engines=(
                        mybir.EngineType.PE,
                        mybir.EngineType.DVE,
                        mybir.EngineType.Activation,
                        mybir.EngineType.SP,
                    ),
                    staggered_reset=staggered,
                )
                if loop_iters > 1
                else contextlib.nullcontext()
            ),
        ):
            pack_sb = cpool.tile([D, PACK_W], F32)
            nc.sync.dma_start(out=pack_sb, in_=pack_d)
            pack16 = pack_sb[:, : PACK_H // 2].bitcast(F16)
            o0 = 0
            xj_sb = pack16[:, o0 : o0 + S]; o0 += S
            w1c_sb = pack16[:, o0 : o0 + D]; o0 += D
            xi_sb = pack16[:, o0 : o0 + I_PER_CORE]; o0 += I_PER_CORE
            w1a_sb = pack16[:, o0 : o0 + D]; o0 += D
            comb_sb = pack16[:, o0 : o0 + 4 * 32]
            b1_sb = pack_sb[:, PACK_W - 2 : PACK_W - 1]
            b2v_sb = pack_sb[:, PACK_W - 1 : PACK_W]

            # precompute borrows one pair-slot from the matmul psum pool:
            # cT in the first bank-half, aT in the second
            pre_ps = mmpool.tile([D, 2 * S], F32, tag="ps2")
            # cT[d_out, j] = sum_k w1c[k, d_out] * xT[k, j]
            nc.tensor.matmul(pre_ps[:, :S], w1c_sb, xj_sb, start=True, stop=True)
            cT_sb = cpool.tile([D, S], dt_h)
            # aT[d_out, i] = sum_k w1a[k, d_out] * xT[k, i]  (+ b1 per partition)
            nc.tensor.matmul(
                pre_ps[:, S : S + I_PER_CORE], w1a_sb, xi_sb, start=True, stop=True
            )
            at_sb = cpool.tile([D, I_PER_CORE], F32)
            if prep_eng == "dve":
                nc.vector.tensor_copy(cT_sb, pre_ps[:, :S])
                nc.vector.tensor_scalar(
                    at_sb,
                    pre_ps[:, S : S + I_PER_CORE],
                    b1_sb,
                    0.0,
                    mybir.AluOpType.add,
                    mybir.AluOpType.bypass,
                )
            else:
                # on ACT (both funcs are in the resident sigmoid_and_others
                # table set); b1 is folded into cT (z = (c+b1) + a), so aT
                # is a plain copy with no bias dependency
                nc.scalar.activation(
                    cT_sb,
                    pre_ps[:, :S],
                    mybir.ActivationFunctionType.Identity,
                    bias=b1_sb,
                )
                nc.scalar.activation(
                    at_sb,
                    pre_ps[:, S : S + I_PER_CORE],
                    mybir.ActivationFunctionType.Copy,
                )

            def drain(t, ps2):
                # sigmoid + store for a finished pair of groups (t, t+1);
                # emitted one pair late so ACT's (stalling) sigmoid sits
                # behind the next pair's h-gen ops in ACT program order.
                o_sb = opool.tile([D, 2 * S], dt_o)
                nc.scalar.activation(o_sb, ps2, sigmoid, bias=b2v_sb)
                if store_eng.startswith("one2"):
                    # single fully-contiguous 256KB store per drain
                    if store_eng == "one2a":
                        eng = nc.scalar
                    elif store_eng == "one2s":
                        eng = nc.sync
                    elif store_eng == "one2h":
                        # early drains on SP (its loads are done by then and
                        # finish long before next iteration's loads); late
                        # drains on ACT so iteration-end stores never block
                        # the next iteration's SP load queue
                        eng = nc.sync if (t // 2) < 4 else nc.scalar
                    else:  # one2: alternate queues per drain
                        eng = nc.sync if (t // 2) % 2 == 0 else nc.scalar
                    eng.dma_start(out=out_d[t // 2], in_=o_sb)
                    return
                # partition p = g*8+h  ->  out[h, (t+u)*16+g, :]
                half = D // 2
                for u in range(2):
                    dst = out_d[:, (t + u) * G : (t + u + 1) * G, :].rearrange(
                        "h g j -> g h j"
                    )
                    src = o_sb[:, u * S : (u + 1) * S]
                    if store_eng == "big":
                        # one full-width store per u-half; alternate engines
                        eng = nc.sync if u == 0 else nc.scalar
                        eng.dma_start(out=dst, in_=src)
                    elif store_eng == "bigsp":
                        nc.sync.dma_start(out=dst, in_=src)
                    else:  # "split": halves across SP + ACT queues
                        nc.sync.dma_start(out=dst[: G // 2], in_=src[:half])
                        nc.scalar.dma_start(out=dst[G // 2 :], in_=src[half:])

            if diag == "pe":
                # PE-pure stream: one static h tile, full matmul schedule
                h_static = cpool.tile([D, 2 * S], dt_h)
                nc.vector.tensor_copy(h_static[:, :S], cT_sb)
                nc.vector.tensor_copy(h_static[:, S:], cT_sb)
                for t in range(0, T, 2):
                    ps2 = mmpool.tile([D, 2 * S], F32, tag="ps2")
                    g_order = [4 * q + r for r in range(4) for q in range(4)]
                    for n, g in enumerate(g_order):
                        q, r = g // 4, g % 4
                        for u in range(2):
                            nc.tensor.matmul(
                                ps2[32 * q : 32 * (q + 1), u * S : (u + 1) * S],
                                comb_sb[:, 32 * r : 32 * (r + 1)],
                                h_static[:, u * S : (u + 1) * S],
                                start=(r == 0),
                                stop=(r == 3),
                                tile_position=(0, 32 * q),
                                skip_group_check=True,
                            )
                    o_sb = opool.tile([D, 2 * S], F32)
                    nc.scalar.activation(o_sb, ps2, sigmoid, bias=b2v_sb)
                    half = D // 2
                    for u in range(2):
                        dst = out_d[:, (t + u) * G : (t + u + 1) * G, :]
                        src = o_sb[:, u * S : (u + 1) * S]
                        nc.sync.dma_start(out=dst[: H // 2], in_=src[:half])
                        nc.scalar.dma_start(out=dst[H // 2 :], in_=src[half:])
            elif diag == "dve":
                # DVE-pure stream: all h-gen ops, no matmul/sigmoid; dump one
                # h tile to out to keep outputs written
                for t in range(0, T, 2):
                    for g in range(G):
                        h2 = hpool.tile([D, 2 * S], dt_h)
                        for u in range(2):
                            i_loc = (t + u) * G + g
                            a_col = at_sb[:, i_loc : i_loc + 1]
                            dst = h2[:, u * S : (u + 1) * S]
                            nc.vector.tensor_scalar(dst, cT_sb, a_col, 0.0, add, amax)
                    o_sb = opool.tile([D, 2 * S], F32)
                    nc.vector.tensor_copy(o_sb, h2)
                    half = D // 2
                    for u in range(2):
                        dst = out_d[:, (t + u) * G : (t + u + 1) * G, :]
                        src = o_sb[:, u * S : (u + 1) * S]
                        nc.sync.dma_start(out=dst[: H // 2], in_=src[:half])
                        nc.scalar.dma_start(out=dst[H // 2 :], in_=src[half:])
            elif not pipe:
                pending = None  # (t, psum tile) awaiting sigmoid+store
                # emission order: r outer, u middle, q fastest -> consecutive
                # matmuls land in 4 different 32-col PE strips (concurrent
                # streaming); a strip's accumulation chain (same q,u across r)
                # recurs only every 8 instructions.
                if mm_order == "ruq":
                    sched = [
                        (4 * q + r, u)
                        for r in range(4)
                        for u in range(2)
                        for q in range(4)
                    ]
                else:  # "rqu": the original order, u innermost
                    sched = [
                        (4 * q + r, u)
                        for r in range(4)
                        for q in range(4)
                        for u in range(2)
                    ]
                for t in range(0, T, 2):
                    # two groups (t, t+1) share one 2-bank PSUM tile: matmul g
                    # covers j 0..511 for row t*16+g and j 512..1023 for row
                    # (t+1)*16+g with the same comb_g weights.
                    ps2 = mmpool.tile([D, 2 * S], F32, tag="ps2")
                    if n_act >= 100:
                        # spread k ACT rows evenly over the 32 entries
                        act_k = n_act - 100
                        act_pos = {(i * 32) // act_k for i in range(act_k)}
                    else:
                        act_pos = None
                    for n, (g, u) in enumerate(sched):
                        q, r = g // 4, g % 4
                        i_loc = (t + u) * G + g
                        a_col = at_sb[:, i_loc : i_loc + 1]
                        if act_pos is not None:
                            on_act = n in act_pos
                        else:
                            # last act_k of the 32 sched entries go to ACT (all
                            # r==3 tail positions when act_k <= 8)
                            act_k = n_act * 2 if n_act < 8 else n_act - 8
                            on_act = n >= 32 - act_k
                        dt_row = F8 if (on_act and act_f8) else dt_h
                        hu = hpool.tile([D, S], dt_row, tag=f"h{u}{'f8' if dt_row is F8 else ''}")
                        if on_act:
                            nc.scalar.activation(hu, cT_sb, relu, bias=a_col)
                        else:
                            nc.vector.tensor_scalar(hu, cT_sb, a_col, 0.0, add, amax)
                        nc.tensor.matmul(
                            ps2[32 * q : 32 * (q + 1), u * S : (u + 1) * S],
                            comb_sb[:, 32 * r : 32 * (r + 1)],
                            hu,
                            start=(r == 0),
                            stop=(r == 3),
                            tile_position=(0, 32 * q),
                            skip_group_check=True,
                        )
                        if n == 3 and pending is not None:
                            drain(*pending)
                            pending = None
                    pending = (t, ps2)
                drain(*pending)
            else:
                # ACT-lookahead pipeline: ACT produces its h rows one pair
                # AHEAD of consumption so its (jittery, sigmoid-interleaved)
                # stream never stalls the PE's in-order consumption; only the
                # DVE remains tightly coupled.
                pending = None
                if mm_order == "ruq":
                    sched = [
                        (4 * q + r, u)
                        for r in range(4)
                        for u in range(2)
                        for q in range(4)
                    ]
                else:
                    sched = [
                        (4 * q + r, u)
                        for r in range(4)
                        for q in range(4)
                        for u in range(2)
                    ]
                act_k = (n_act - 100) if n_act >= 100 else max(0, n_act - 8)
                # offset by 2 so the PE's first consumption (and the last
                # pair's tail) is never ACT-gated
                act_pos = (
                    sorted({min(2 + (i * 30) // act_k, 31) for i in range(act_k)})
                    if act_k
                    else []
                )
                act_set = set(act_pos)
                act_tiles = {}
                # last pair runs u-major so its u=0 PSUM half finishes at
                # entry 15 and drains while the u=1 matmuls still run
                sched_last = [
                    (4 * q + r, u) for u in range(2) for r in range(4) for q in range(4)
                ]

                def sched_for(t):
                    return sched_last if t == T - 2 else sched

                def act_produce(t):
                    sch = sched_for(t)
                    for idx, n in enumerate(act_pos):
                        g, u = sch[n]
                        i_loc = (t + u) * G + g
                        ha = hpool.tile(
                            [D, S], dt_h, tag=f"A{idx}", bufs=a_bufs
                        )
                        nc.scalar.activation(
                            ha, cT_sb, relu, bias=at_sb[:, i_loc : i_loc + 1]
                        )
                        act_tiles[(t, n)] = ha

                def drain_half(t, u, ps2):
                    oh = opool.tile([D, S], dt_o, tag="oh")
                    nc.scalar.activation(
                        oh, ps2[:, u * S : (u + 1) * S], sigmoid, bias=b2v_sb
                    )
                    nc.scalar.dma_start(
                        out=out_d[t // 2, :, u * S : (u + 1) * S], in_=oh
                    )

                act_produce(0)
                for t in range(0, T, 2):
                    ps2 = mmpool.tile([D, 2 * S], F32, tag="ps2")
                    for n, (g, u) in enumerate(sched_for(t)):
                        q, r = g // 4, g % 4
                        i_loc = (t + u) * G + g
                        if n in act_set:
                            hu = act_tiles.pop((t, n))
                        else:
                            hu = hpool.tile([D, S], dt_h, tag=f"h{u}")
                            nc.vector.tensor_scalar(
                                hu,
                                cT_sb,
                                at_sb[:, i_loc : i_loc + 1],
                                0.0,
                                add,
                                amax,
                            )
                        nc.tensor.matmul(
                            ps2[32 * q : 32 * (q + 1), u * S : (u + 1) * S],
                            comb_sb[:, 32 * r : 32 * (r + 1)],
                            hu,
                            start=(r == 0),
                            stop=(r == 3),
                            tile_position=(0, 32 * q),
                            skip_group_check=True,
                        )
                        if n == 3 and pending is not None:
                            drain(*pending)
                            pending = None
                        if n == 5 and t + 2 < T:
                            act_produce(t + 2)
                        if t == T - 2 and n == 15:
                            drain_half(t, 0, ps2)
                    pending = (t, ps2)
                drain_half(T - 2, 1, pending[1])

    nc.compile()
    # Activation-table cleanup: the table pass puts LoadActFuncSet(0) (relu
    # set) AND LoadActFuncSet(2) (sigmoid_and_others, which also contains
    # Relu) inside the loop body — 2.6us of ACT per iteration. Set 2 serves
    # every activation here (verified bit-identical), so drop the set-0
    # loads and hoist the set-2 load into the preceding block so it runs
    # once instead of per iteration.
    blocks = nc.m.functions[0].blocks
    for bi, b in enumerate(blocks):
        b.instructions[:] = [
            i
            for i in b.instructions
            if not (isinstance(i, mybir.InstLoadActFuncSet) and i.act_func_set_id == 0)
        ]
        if bi > 0:
            hoist = [
                i for i in b.instructions if isinstance(i, mybir.InstLoadActFuncSet)
            ]
            if hoist:
                b.instructions[:] = [
                    i
                    for i in b.instructions
                    if not isinstance(i, mybir.InstLoadActFuncSet)
                ]
                for i in reversed(hoist):
                    blocks[bi - 1].instructions.insert(0, i)
    return nc


def _host_prep(node_features, w1, b1, w2, b2):
    """Shared (per-core-replicated) small tensors + per-core input maps."""
    w1a = np.ascontiguousarray(w1[:D])  # [D, D] == lhsT for aT
    w1c = np.ascontiguousarray(w1[D:])  # [D, D] == lhsT for cT
    b1c = np.ascontiguousarray(b1.reshape(D, 1))
    # psum partition p = g*8 + h; col-group q = g//4 covers partitions
    # [32q, 32q+32); weight tile r = g%4 has w2 in columns [8r, 8r+8)
    comb = np.zeros((D, 4, 32), np.float32)
    for r in range(4):
        comb[:, r, r * H : (r + 1) * H] = w2
    comb = np.ascontiguousarray(comb.reshape(D, 4 * 32).astype(F16_NP))
    b2v = np.ascontiguousarray(np.tile(b2, G).reshape(D, 1))

    in_maps = []
    for k in range(N_CORES):
        b = k // (N_CORES // B)
        i0 = (k % (N_CORES // B)) * I_PER_CORE
        xT = np.ascontiguousarray(node_features[b].T)  # [D, S]
        pack16 = np.concatenate(
            [
                xT.astype(F16_NP),
                w1c.astype(F16_NP),
                xT[:, i0 : i0 + I_PER_CORE].astype(F16_NP),
                w1a.astype(F16_NP),
                comb,
            ],
            axis=1,
        )
        pack = np.concatenate(
            [pack16.view(np.float32), b1c, b2v], axis=1
        )
        in_maps.append({"pack": np.ascontiguousarray(pack)})
    return in_maps


def _gather(results):
    out = np.empty((B, H, S, S), np.float32)
    for k in range(N_CORES):
        b = k // (N_CORES // B)
        i0 = (k % (N_CORES // B)) * I_PER_CORE
        arr = results[k]["out"]
        if arr.dtype != np.float32:
            arr = arr.astype(np.float32)
        if arr.shape[0] == T // 2:  # raw drain-major layout (one2 stores)
            # arr[d, g*8+h, u*512+j] -> out[b, h, i0 + d*32+u*16+g, j]
            a5 = arr.reshape(T // 2, G, H, 2, S)  # [d, g, h, u, j]
            out[b, :, i0 : i0 + I_PER_CORE, :] = (
                a5.transpose(2, 0, 3, 1, 4).reshape(H, I_PER_CORE, S)
            )
        else:
            out[b, :, i0 : i0 + I_PER_CORE, :] = arr
    return out


def _build_jit(nc):
    """Single cached jit around the bass_exec custom call (the stock
    run_bass_kernel_spmd path re-traces/jits on every invocation)."""
    import jax
    from jax.sharding import Mesh, PartitionSpec

    try:
        from jax.experimental.shard_map import shard_map
    except ImportError:
        from jax.sharding import shard_map

    from concourse.bass2jax import (
        _bass_exec_p,
        install_neuronx_cc_hook,
        partition_id_tensor,
    )

    install_neuronx_cc_hook()
    partition_name = nc.partition_id_tensor.name if nc.partition_id_tensor else None
    in_names, out_names, out_avals, zero_outs = [], [], [], []
    for alloc in nc.m.functions[0].allocations:
        if not isinstance(alloc, mybir.MemoryLocationSet):
            continue
        name = alloc.memorylocations[0].name
        if alloc.kind == "ExternalInput":
            if name != partition_name:
                in_names.append(name)
        elif alloc.kind == "ExternalOutput":
            shape = tuple(alloc.tensor_shape)
            np_dt = mybir.dt.np(alloc.dtype)
            out_avals.append(jax.core.ShapedArray(shape, np_dt))
            out_names.append(name)
            zero_outs.append(np.zeros(shape, np_dt))
    n_params = len(in_names)
    all_in_names = list(in_names) + list(out_names)
    if partition_name is not None:
        all_in_names.append(partition_name)

    def _body(*args):
        operands = list(args)
        if partition_name is not None:
            operands.append(partition_id_tensor())
        return tuple(
            _bass_exec_p.bind(
                *operands,
                out_avals=tuple(out_avals),
                in_names=tuple(all_in_names),
                out_names=tuple(out_names),
                lowering_input_output_aliases=(),
                sim_require_finite=True,
                sim_require_nnan=True,
                nc=nc,
            )
        )

    devices = jax.devices()[:N_CORES]
    mesh = Mesh(np.asarray(devices), ("core",))
    n_outs = len(out_names)
    sharded = jax.jit(
        shard_map(
            _body,
            mesh=mesh,
            in_specs=(PartitionSpec("core"),) * (n_params + n_outs),
            out_specs=(PartitionSpec("core"),) * n_outs,
            check_rep=False,
        ),
        # no donation: the kernel writes every output element, so the zero
        # operand buffers can live on device and be reused across calls
        keep_unused=True,
    )
    return sharded, in_names, out_names, zero_outs


def _run(in_maps):
    if "nc" not in _CACHE:
        _CACHE["nc"] = _build_nc()
        _CACHE["jit"] = _build_jit(_CACHE["nc"])
    sharded, in_names, out_names, zero_outs = _CACHE["jit"]
    concat_in = [
        np.concatenate([np.asarray(in_maps[c][n]) for c in range(N_CORES)], axis=0)
        for n in in_names
    ]
    if "zeros_dev" not in _CACHE:
        import jax

        _CACHE["zeros_dev"] = [
            jax.device_put(np.zeros((N_CORES * z.shape[0], *z.shape[1:]), z.dtype))
            for z in zero_outs
        ]
    out_arrs = sharded(*concat_in, *_CACHE["zeros_dev"])
    # outputs come back concatenated on axis 0 (N_CORES * dim0, ...)
    split = []
    for i, name in enumerate(out_names):
        arr = np.asarray(out_arrs[i])
        split.append(arr.reshape(N_CORES, arr.shape[0] // N_CORES, *arr.shape[1:]))
    return [
        {name: split[i][c] for i, name in enumerate(out_names)}
        for c in range(N_CORES)
    ]


def kernel(node_features, w1, b1, w2, b2):
    node_features = np.asarray(node_features, np.float32)
    w1 = np.asarray(w1, np.float32)
    b1 = np.asarray(b1, np.float32)
    w2 = np.asarray(w2, np.float32)
    b2 = np.asarray(b2, np.float32)
    in_maps = _host_prep(node_features, w1, b1, w2, b2)
    results = _run(in_maps)
    return _gather(results)



# revision 39
# speedup vs baseline: 1.0965x; 1.0303x over previous
"""Bass/Trainium2 kernel for nn_DynamicEdgeWeights.

Math (B=4, S=512, D=128, H=8):
    a = x @ w1[:D]; c = x @ w1[D:]
    h[b,i,j,:] = relu(a[b,i,:] + c[b,j,:] + b1)
    out[b,h,i,j] = sigmoid(sum_d h[b,i,j,d] * w2[d,h] + b2[h])

Device strategy (per core; 8 cores, core k -> batch k//2, i-rows [(k%2)*256, +256)):
  - cT[d, j] = (x[b] @ w1c).T (+b1 folded in) and aT[d, i] = (x[b] @ w1a).T
    via two PE matmuls on pre-transposed fp16 x (host packs ALL constants
    into ONE [128, 578] f32 dram tensor -> a single head DMA; the For_i
    reset block is an all-engine barrier, so head DMA-issue count and
    transfer size are paid every iteration).
  - per query row i: one fused relu(cT + aT[:, i]) producing h_i [128d, 512j].
    Measured engine rates: DVE tensor_scalar fp16 303.5ns/row (4x mode,
    the hard floor: 256 ops minimum since the per-partition scalar is
    per-row), ACT activation 632ns/row, sigmoid [128,1024] 1213ns.
    Split: 24 rows/pair DVE + 8/pair ACT (n_act=108), with ACT producing
    its rows one pair AHEAD (lookahead pipeline) so its jitter never
    stalls the PE's in-order consumption.
  - second matmul uses "comb" weights: 16 query rows share one PSUM bank.
    comb_g [128, 128] has w2[:, h] in column h*16+g, zeros elsewhere; 16
    accumulating matmuls put e-pre for (16 i x 8 h) on 128 PSUM partitions.
    4-strip col tiling streams ~2.4x concurrent (92.9ns/matmul measured).
  - groups are processed in pairs sharing a 2-bank PSUM tile; one full-width
    [128, 1024] sigmoid (ACT, bias=b2 broadcast) -> fp16 SBUF -> one
    contiguous store per pair. The LAST pair runs u-major and drains in
    halves so the tail (which the barrier serializes) is ~1.4us shorter.
"""

import os
import sys

for _p in ("/opt/trn_rl_repo", "/root/.axon_site/_ro/trn_rl_repo"):
    if os.path.isdir(_p) and _p not in sys.path:
        sys.path.insert(0, _p)
        break

import numpy as np
import ml_dtypes  # noqa: F401  (registers bfloat16 dtype)

import concourse.bass as bass  # noqa: F401  (registers types)
import concourse.mybir as mybir
from concourse import bacc
from concourse.tile import TileContext

B, S, D, H = 4, 512, 128, 8
N_CORES = 8
I_PER_CORE = (B * S) // N_CORES  # 256
G = 16  # query rows packed per PSUM bank
T = I_PER_CORE // G  # 16 groups per core
# ACT h-gen share: values >= 8 mean (n_act - 8) of the last 8 sched entries
# per pair go to ScalarE (rest DVE); 13 -> 5 of 32 rows per pair on ACT.
# Values 100+k mean k rows per pair spread EVENLY through the 32 sched
# entries (measured balance: DVE 303.5ns/row vs ACT ~630-730ns/row +
# 1213ns sigmoid per pair -> optimum 8/pair measured).
N_ACT = 108
STORE_ENG = "one2s"  # all drain stores issued from SP (ACT queue stays clear)

F32 = mybir.dt.float32
F16 = mybir.dt.float16  # h-path dtype: full PE rate (fp32 streams at 1/4 rate)
F8 = mybir.dt.float8e4  # optional ACT-row dtype (ACT writes 1-byte faster)
F16_NP = "float16"

_CACHE: dict = {}


def _build_nc(loop_iters: int = 1, dt_h=F16, n_act=N_ACT, h_bufs=8, o_bufs=8, mm_bufs=4, staggered=False, diag=None, store_eng=None, mm_order="rqu", act_f8=False, store_f16=True, c_bufs=2, pipe=True, prep_eng="act", a_bufs=3, quad=False):
    if store_eng is None:
        store_eng = STORE_ENG
    """Build the single-core Bass program (identical across the 8 cores).

    loop_iters > 1 wraps the whole compute in an on-device For_i repeat —
    used only for steady-state timing (one dispatch, N executions).
    """
    nc = bacc.Bacc(
        "TRN2",
        target_bir_lowering=False,
        debug=False,
        enable_asserts=False,
        num_devices=N_CORES,
    )

    # single packed constants tensor, mostly fp16:
    # [xj | w1c | xi | w1a | comb]:f16  ++  [b1 | b2v]:f32 (bit-packed into
    # one f32 row-major buffer) -> ONE head DMA instead of 7 serialized
    # ~630ns DGE issues; the For_i reset barrier makes the head serial, so
    # both issue count and transfer size are on the critical path.
    PACK_H = S + D + I_PER_CORE + D + 4 * 32  # fp16 columns
    PACK_W = PACK_H // 2 + 2
    pack_d = nc.dram_tensor("pack", (D, PACK_W), F32, kind="ExternalInput").ap()
    dt_o = F16 if store_f16 else F32
    if store_eng.startswith("one2"):
        # raw drain-major layout: [drain, (g,h) partition, (u,j)]; host
        # un-permutes in _gather. Stores are fully contiguous.
        out_d = nc.dram_tensor(
            "out", (T // 2, D, 2 * S), dt_o, kind="ExternalOutput"
        ).ap()
    else:
        out_d = nc.dram_tensor(
            "out", (H, I_PER_CORE, S), dt_o, kind="ExternalOutput"
        ).ap()

    relu = mybir.ActivationFunctionType.Relu
    sigmoid = mybir.ActivationFunctionType.Sigmoid
    add = mybir.AluOpType.add
    amax = mybir.AluOpType.max

    import contextlib

    with TileContext(nc) as tc:
        with (
            # bufs=2: next iteration's loads + cT/aT precompute overlap this
            # iteration's tail instead of WAR-blocking on the single buffer
            tc.tile_pool(name="const", bufs=c_bufs) as cpool,
            tc.tile_pool(name="h", bufs=h_bufs) as hpool,
            tc.tile_pool(name="o", bufs=o_bufs) as opool,
            tc.tile_pool(name="mm", bufs=mm_bufs, space="PSUM") as mmpool,
            (
                tc.For_i(
                    0,
                    loop_iters,
                    1,
                    hint_engines=(
                        mybir.EngineType.PE,
                        mybir.EngineType.DVE,
                        mybir.EngineType.Activation,
                        mybir.EngineType.SP,
                    ),
                    staggered_reset=staggered,
                )
                if loop_iters > 1
                else contextlib.nullcontext()
            ),
        ):
            pack_sb = cpool.tile([D, PACK_W], F32)
            nc.sync.dma_start(out=pack_sb, in_=pack_d)
            pack16 = pack_sb[:, : PACK_H // 2].bitcast(F16)
            o0 = 0
            xj_sb = pack16[:, o0 : o0 + S]; o0 += S
            w1c_sb = pack16[:, o0 : o0 + D]; o0 += D
            xi_sb = pack16[:, o0 : o0 + I_PER_CORE]; o0 += I_PER_CORE
            w1a_sb = pack16[:, o0 : o0 + D]; o0 += D
            comb_sb = pack16[:, o0 : o0 + 4 * 32]
            b1_sb = pack_sb[:, PACK_W - 2 : PACK_W - 1]
            b2v_sb = pack_sb[:, PACK_W - 1 : PACK_W]

            # precompute borrows one pair-slot from the matmul psum pool:
            # cT in the first bank-half, aT in the second
            if pipe and diag is None and quad:
                # share the quad psum rotation (2 x [D,4S] tiles = all 8 banks)
                pre_full = mmpool.tile(
                    [D, 4 * S], F32, tag="ps4", bufs=2, name="pre_full"
                )
                pre_ps = pre_full[:, : 2 * S]
            else:
                pre_ps = mmpool.tile([D, 2 * S], F32, tag="ps2")
            # cT[d_out, j] = sum_k w1c[k, d_out] * xT[k, j]
            nc.tensor.matmul(pre_ps[:, :S], w1c_sb, xj_sb, start=True, stop=True)
            cT_sb = cpool.tile([D, S], dt_h)
            # aT[d_out, i] = sum_k w1a[k, d_out] * xT[k, i]  (+ b1 per partition)
            nc.tensor.matmul(
                pre_ps[:, S : S + I_PER_CORE], w1a_sb, xi_sb, start=True, stop=True
            )
            at_sb = cpool.tile([D, I_PER_CORE], F32)
            if prep_eng == "dve":
                nc.vector.tensor_copy(cT_sb, pre_ps[:, :S])
                nc.vector.tensor_scalar(
                    at_sb,
                    pre_ps[:, S : S + I_PER_CORE],
                    b1_sb,
                    0.0,
                    mybir.AluOpType.add,
                    mybir.AluOpType.bypass,
                )
            else:
                # on ACT (both funcs are in the resident sigmoid_and_others
                # table set); b1 is folded into cT (z = (c+b1) + a), so aT
                # is a plain copy with no bias dependency
                nc.scalar.activation(
                    cT_sb,
                    pre_ps[:, :S],
                    mybir.ActivationFunctionType.Identity,
                    bias=b1_sb,
                )
                nc.scalar.activation(
                    at_sb,
                    pre_ps[:, S : S + I_PER_CORE],
                    mybir.ActivationFunctionType.Copy,
                )

            def drain(t, ps2):
                # sigmoid + store for a finished pair of groups (t, t+1);
                # emitted one pair late so ACT's (stalling) sigmoid sits
                # behind the next pair's h-gen ops in ACT program order.
                o_sb = opool.tile([D, 2 * S], dt_o)
                nc.scalar.activation(o_sb, ps2, sigmoid, bias=b2v_sb)
                if store_eng.startswith("one2"):
                    # single fully-contiguous 256KB store per drain
                    if store_eng == "one2a":
                        eng = nc.scalar
                    elif store_eng == "one2s":
                        eng = nc.sync
                    elif store_eng == "one2h":
                        # early drains on SP (its loads are done by then and
                        # finish long before next iteration's loads); late
                        # drains on ACT so iteration-end stores never block
                        # the next iteration's SP load queue
                        eng = nc.sync if (t // 2) < 4 else nc.scalar
                    else:  # one2: alternate queues per drain
                        eng = nc.sync if (t // 2) % 2 == 0 else nc.scalar
                    eng.dma_start(out=out_d[t // 2], in_=o_sb)
                    return
                # partition p = g*8+h  ->  out[h, (t+u)*16+g, :]
                half = D // 2
                for u in range(2):
                    dst = out_d[:, (t + u) * G : (t + u + 1) * G, :].rearrange(
                        "h g j -> g h j"
                    )
                    src = o_sb[:, u * S : (u + 1) * S]
                    if store_eng == "big":
                        # one full-width store per u-half; alternate engines
                        eng = nc.sync if u == 0 else nc.scalar
                        eng.dma_start(out=dst, in_=src)
                    elif store_eng == "bigsp":
                        nc.sync.dma_start(out=dst, in_=src)
                    else:  # "split": halves across SP + ACT queues
                        nc.sync.dma_start(out=dst[: G // 2], in_=src[:half])
                        nc.scalar.dma_start(out=dst[G // 2 :], in_=src[half:])

            if diag == "pe":
                # PE-pure stream: one static h tile, full matmul schedule
                h_static = cpool.tile([D, 2 * S], dt_h)
                nc.vector.tensor_copy(h_static[:, :S], cT_sb)
                nc.vector.tensor_copy(h_static[:, S:], cT_sb)
                for t in range(0, T, 2):
                    ps2 = mmpool.tile([D, 2 * S], F32, tag="ps2")
                    g_order = [4 * q + r for r in range(4) for q in range(4)]
                    for n, g in enumerate(g_order):
                        q, r = g // 4, g % 4
                        for u in range(2):
                            nc.tensor.matmul(
                                ps2[32 * q : 32 * (q + 1), u * S : (u + 1) * S],
                                comb_sb[:, 32 * r : 32 * (r + 1)],
                                h_static[:, u * S : (u + 1) * S],
                                start=(r == 0),
                                stop=(r == 3),
                                tile_position=(0, 32 * q),
                                skip_group_check=True,
                            )
                    o_sb = opool.tile([D, 2 * S], F32)
                    nc.scalar.activation(o_sb, ps2, sigmoid, bias=b2v_sb)
                    half = D // 2
                    for u in range(2):
                        dst = out_d[:, (t + u) * G : (t + u + 1) * G, :]
                        src = o_sb[:, u * S : (u + 1) * S]
                        nc.sync.dma_start(out=dst[: H // 2], in_=src[:half])
                        nc.scalar.dma_start(out=dst[H // 2 :], in_=src[half:])
            elif diag == "dve":
                # DVE-pure stream: all h-gen ops, no matmul/sigmoid; dump one
                # h tile to out to keep outputs written
                for t in range(0, T, 2):
                    for g in range(G):
                        h2 = hpool.tile([D, 2 * S], dt_h)
                        for u in range(2):
                            i_loc = (t + u) * G + g
                            a_col = at_sb[:, i_loc : i_loc + 1]
                            dst = h2[:, u * S : (u + 1) * S]
                            nc.vector.tensor_scalar(dst, cT_sb, a_col, 0.0, add, amax)
                    o_sb = opool.tile([D, 2 * S], F32)
                    nc.vector.tensor_copy(o_sb, h2)
                    half = D // 2
                    for u in range(2):
                        dst = out_d[:, (t + u) * G : (t + u + 1) * G, :]
                        src = o_sb[:, u * S : (u + 1) * S]
                        nc.sync.dma_start(out=dst[: H // 2], in_=src[:half])
                        nc.scalar.dma_start(out=dst[H // 2 :], in_=src[half:])
            elif not pipe:
                pending = None  # (t, psum tile) awaiting sigmoid+store
                # emission order: r outer, u middle, q fastest -> consecutive
                # matmuls land in 4 different 32-col PE strips (concurrent
                # streaming); a strip's accumulation chain (same q,u across r)
                # recurs only every 8 instructions.
                if mm_order == "ruq":
                    sched = [
                        (4 * q + r, u)
                        for r in range(4)
                        for u in range(2)
                        for q in range(4)
                    ]
                else:  # "rqu": the original order, u innermost
                    sched = [
                        (4 * q + r, u)
                        for r in range(4)
                        for q in range(4)
                        for u in range(2)
                    ]
                for t in range(0, T, 2):
                    # two groups (t, t+1) share one 2-bank PSUM tile: matmul g
                    # covers j 0..511 for row t*16+g and j 512..1023 for row
                    # (t+1)*16+g with the same comb_g weights.
                    ps2 = mmpool.tile([D, 2 * S], F32, tag="ps2")
                    if n_act >= 100:
                        # spread k ACT rows evenly over the 32 entries
                        act_k = n_act - 100
                        act_pos = {(i * 32) // act_k for i in range(act_k)}
                    else:
                        act_pos = None
                    for n, (g, u) in enumerate(sched):
                        q, r = g // 4, g % 4
                        i_loc = (t + u) * G + g
                        a_col = at_sb[:, i_loc : i_loc + 1]
                        if act_pos is not None:
                            on_act = n in act_pos
                        else:
                            # last act_k of the 32 sched entries go to ACT (all
                            # r==3 tail positions when act_k <= 8)
                            act_k = n_act * 2 if n_act < 8 else n_act - 8
                            on_act = n >= 32 - act_k
                        dt_row = F8 if (on_act and act_f8) else dt_h
                        hu = hpool.tile([D, S], dt_row, tag=f"h{u}{'f8' if dt_row is F8 else ''}")
                        if on_act:
                            nc.scalar.activation(hu, cT_sb, relu, bias=a_col)
                        else:
                            nc.vector.tensor_scalar(hu, cT_sb, a_col, 0.0, add, amax)
                        nc.tensor.matmul(
                            ps2[32 * q : 32 * (q + 1), u * S : (u + 1) * S],
                            comb_sb[:, 32 * r : 32 * (r + 1)],
                            hu,
                            start=(r == 0),
                            stop=(r == 3),
                            tile_position=(0, 32 * q),
                            skip_group_check=True,
                        )
                        if n == 3 and pending is not None:
                            drain(*pending)
                            pending = None
                    pending = (t, ps2)
                drain(*pending)
            else:
                # ACT-lookahead pipeline: ACT produces its h rows one pair
                # AHEAD of consumption so its (jittery, sigmoid-interleaved)
                # stream never stalls the PE's in-order consumption; only the
                # DVE remains tightly coupled.
                pending = None
                if mm_order == "ruq":
                    sched = [
                        (4 * q + r, u)
                        for r in range(4)
                        for u in range(2)
                        for q in range(4)
                    ]
                else:
                    sched = [
                        (4 * q + r, u)
                        for r in range(4)
                        for q in range(4)
                        for u in range(2)
                    ]
                act_k = (n_act - 100) if n_act >= 100 else max(0, n_act - 8)
                # offset by 2 so the PE's first consumption (and the last
                # pair's tail) is never ACT-gated
                act_pos = (
                    sorted({min(2 + (i * 30) // act_k, 31) for i in range(act_k)})
                    if act_k
                    else []
                )
                act_set = set(act_pos)
                act_tiles = {}
                # last pair runs u-major so its u=0 PSUM half finishes at
                # entry 15 and drains while the u=1 matmuls still run
                sched_last = [
                    (4 * q + r, u) for u in range(2) for r in range(4) for q in range(4)
                ]

                def sched_for(t):
                    return sched_last if t == T - 2 else sched

                def act_produce(t):
                    sch = sched_for(t)
                    for idx, n in enumerate(act_pos):
                        g, u = sch[n]
                        i_loc = (t + u) * G + g
                        ha = hpool.tile(
                            [D, S], dt_h, tag=f"A{idx}", bufs=a_bufs
                        )
                        nc.scalar.activation(
                            ha, cT_sb, relu, bias=at_sb[:, i_loc : i_loc + 1]
                        )
                        act_tiles[(t, n)] = ha

                def drain_half(t, u, ps2):
                    oh = opool.tile([D, S], dt_o, tag="oh")
                    nc.scalar.activation(
                        oh, ps2[:, u * S : (u + 1) * S], sigmoid, bias=b2v_sb
                    )
                    # issue from SP: it is idle at iteration end, and keeping
                    # the store off ACT's queue shortens ACT's path to the
                    # For_i reset barrier
                    nc.sync.dma_start(
                        out=out_d[t // 2, :, u * S : (u + 1) * S], in_=oh
                    )

                def drain_quad(qt):
                    # one [128, 2048] sigmoid + one contiguous store for two
                    # pair-drains (pairs 2*qt, 2*qt+1)
                    o4 = opool.tile([D, 4 * S], dt_o, tag="o4", bufs=2)
                    nc.scalar.activation(o4, quads[qt], sigmoid, bias=b2v_sb)
                    dst = out_d[2 * qt : 2 * qt + 2].rearrange("p d c -> d p c")
                    nc.scalar.dma_start(out=dst, in_=o4)

                def drain_pair_of_quad(qt, half):
                    o_sb = opool.tile([D, 2 * S], dt_o)
                    nc.scalar.activation(
                        o_sb,
                        quads[qt][:, half * 2 * S : (half + 1) * 2 * S],
                        sigmoid,
                        bias=b2v_sb,
                    )
                    eng = nc.sync if (2 * qt + half) < 4 else nc.scalar
                    eng.dma_start(out=out_d[2 * qt + half], in_=o_sb)

                quads = {}
                act_produce(0)
                for t in range(0, T, 2):
                    p = t // 2
                    if quad:
                        if p % 2 == 0:
                            quads[p // 2] = mmpool.tile(
                                [D, 4 * S], F32, tag="ps4", bufs=2, name="quad"
                            )
                        ps2 = quads[p // 2][
                            :, (p % 2) * 2 * S : (p % 2 + 1) * 2 * S
                        ]
                    else:
                        ps2 = mmpool.tile([D, 2 * S], F32, tag="ps2")
                    for n, (g, u) in enumerate(sched_for(t)):
                        q, r = g // 4, g % 4
                        i_loc = (t + u) * G + g
                        if n in act_set:
                            hu = act_tiles.pop((t, n))
                        else:
                            hu = hpool.tile([D, S], dt_h, tag=f"h{u}")
                            nc.vector.tensor_scalar(
                                hu,
                                cT_sb,
                                at_sb[:, i_loc : i_loc + 1],
                                0.0,
                                add,
                                amax,
                            )
                        nc.tensor.matmul(
                            ps2[32 * q : 32 * (q + 1), u * S : (u + 1) * S],
                            comb_sb[:, 32 * r : 32 * (r + 1)],
                            hu,
                            start=(r == 0),
                            stop=(r == 3),
                            tile_position=(0, 32 * q),
                            skip_group_check=True,
                        )
                        if n == 3:
                            if quad:
                                # drain completed quad (pairs of quad p//2-1)
                                if p % 2 == 0 and p >= 2 and p // 2 - 1 < (T // 4) - 1:
                                    drain_quad(p // 2 - 1)
                                elif p == T // 2 - 1:
                                    # last pair: drain the previous pair
                                    # (first half of the last quad) alone
                                    drain_pair_of_quad((T // 4) - 1, 0)
                            elif pending is not None:
                                drain(*pending)
                                pending = None
                        if n == 5 and t + 2 < T:
                            act_produce(t + 2)
                        if t == T - 2 and n == 15:
                            drain_half(t, 0, ps2)
                    pending = (t, ps2)
                drain_half(T - 2, 1, pending[1])

    nc.compile()
    # Activation-table cleanup: the table pass puts LoadActFuncSet(0) (relu
    # set) AND LoadActFuncSet(2) (sigmoid_and_others, which also contains
    # Relu) inside the loop body — 2.6us of ACT per iteration. Set 2 serves
    # every activation here (verified bit-identical), so drop the set-0
    # loads and hoist the set-2 load into the preceding block so it runs
    # once instead of per iteration.
    blocks = nc.m.functions[0].blocks
    for bi, b in enumerate(blocks):
        b.instructions[:] = [
            i
            for i in b.instructions
            if not (isinstance(i, mybir.InstLoadActFuncSet) and i.act_func_set_id == 0)
        ]
        if bi > 0:
            hoist = [
                i for i in b.instructions if isinstance(i, mybir.InstLoadActFuncSet)
            ]
            if hoist:
                b.instructions[:] = [
                    i
                    for i in b.instructions
                    if not isinstance(i, mybir.InstLoadActFuncSet)
                ]
                for i in reversed(hoist):
                    blocks[bi - 1].instructions.insert(0, i)
    return nc


def _host_prep(node_features, w1, b1, w2, b2):
    """Shared (per-core-replicated) small tensors + per-core input maps."""
    w1a = np.ascontiguousarray(w1[:D])  # [D, D] == lhsT for aT
    w1c = np.ascontiguousarray(w1[D:])  # [D, D] == lhsT for cT
    b1c = np.ascontiguousarray(b1.reshape(D, 1))
    # psum partition p = g*8 + h; col-group q = g//4 covers partitions
    # [32q, 32q+32); weight tile r = g%4 has w2 in columns [8r, 8r+8)
    comb = np.zeros((D, 4, 32), np.float32)
    for r in range(4):
        comb[:, r, r * H : (r + 1) * H] = w2
    comb = np.ascontiguousarray(comb.reshape(D, 4 * 32).astype(F16_NP))
    b2v = np.ascontiguousarray(np.tile(b2, G).reshape(D, 1))

    in_maps = []
    for k in range(N_CORES):
        b = k // (N_CORES // B)
        i0 = (k % (N_CORES // B)) * I_PER_CORE
        xT = np.ascontiguousarray(node_features[b].T)  # [D, S]
        pack16 = np.concatenate(
            [
                xT.astype(F16_NP),
                w1c.astype(F16_NP),
                xT[:, i0 : i0 + I_PER_CORE].astype(F16_NP),
                w1a.astype(F16_NP),
                comb,
            ],
            axis=1,
        )
        pack = np.concatenate(
            [pack16.view(np.float32), b1c, b2v], axis=1
        )
        in_maps.append({"pack": np.ascontiguousarray(pack)})
    return in_maps


def _gather(results):
    out = np.empty((B, H, S, S), np.float32)
    for k in range(N_CORES):
        b = k // (N_CORES // B)
        i0 = (k % (N_CORES // B)) * I_PER_CORE
        arr = results[k]["out"]
        if arr.dtype != np.float32:
            arr = arr.astype(np.float32)
        if arr.shape[0] == T // 2:  # raw drain-major layout (one2 stores)
            # arr[d, g*8+h, u*512+j] -> out[b, h, i0 + d*32+u*16+g, j]
            a5 = arr.reshape(T // 2, G, H, 2, S)  # [d, g, h, u, j]
            out[b, :, i0 : i0 + I_PER_CORE, :] = (
                a5.transpose(2, 0, 3, 1, 4).reshape(H, I_PER_CORE, S)
            )
        else:
            out[b, :, i0 : i0 + I_PER_CORE, :] = arr
    return out


def _build_jit(nc):
    """Single cached jit around the bass_exec custom call (the stock
    run_bass_kernel_spmd path re-traces/jits on every invocation)."""
    import jax
    from jax.sharding import Mesh, PartitionSpec

    try:
        from jax.experimental.shard_map import shard_map
    except ImportError:
        from jax.sharding import shard_map

    from concourse.bass2jax import (
        _bass_exec_p,
        install_neuronx_cc_hook,
        partition_id_tensor,
    )

    install_neuronx_cc_hook()
    partition_name = nc.partition_id_tensor.name if nc.partition_id_tensor else None
    in_names, out_names, out_avals, zero_outs = [], [], [], []
    for alloc in nc.m.functions[0].allocations:
        if not isinstance(alloc, mybir.MemoryLocationSet):
            continue
        name = alloc.memorylocations[0].name
        if alloc.kind == "ExternalInput":
            if name != partition_name:
                in_names.append(name)
        elif alloc.kind == "ExternalOutput":
            shape = tuple(alloc.tensor_shape)
            np_dt = mybir.dt.np(alloc.dtype)
            out_avals.append(jax.core.ShapedArray(shape, np_dt))
            out_names.append(name)
            zero_outs.append(np.zeros(shape, np_dt))
    n_params = len(in_names)
    all_in_names = list(in_names) + list(out_names)
    if partition_name is not None:
        all_in_names.append(partition_name)

    def _body(*args):
        operands = list(args)
        if partition_name is not None:
            operands.append(partition_id_tensor())
        return tuple(
            _bass_exec_p.bind(
                *operands,
                out_avals=tuple(out_avals),
                in_names=tuple(all_in_names),
                out_names=tuple(out_names),
                lowering_input_output_aliases=(),
                sim_require_finite=True,
                sim_require_nnan=True,
                nc=nc,
            )
        )

    devices = jax.devices()[:N_CORES]
    mesh = Mesh(np.asarray(devices), ("core",))
    n_outs = len(out_names)
    sharded = jax.jit(
        shard_map(
            _body,
            mesh=mesh,
            in_specs=(PartitionSpec("core"),) * (n_params + n_outs),
            out_specs=(PartitionSpec("core"),) * n_outs,
            check_rep=False,
        ),
        # no donation: the kernel writes every output element, so the zero
        # operand buffers can live on device and be reused across calls
        keep_unused=True,
    )
    return sharded, in_names, out_names, zero_outs


def _run(in_maps):
    if "nc" not in _CACHE:
        _CACHE["nc"] = _build_nc()
        _CACHE["jit"] = _build_jit(_CACHE["nc"])
    sharded, in_names, out_names, zero_outs = _CACHE["jit"]
    concat_in = [
        np.concatenate([np.asarray(in_maps[c][n]) for c in range(N_CORES)], axis=0)
        for n in in_names
    ]
    if "zeros_dev" not in _CACHE:
        import jax

        _CACHE["zeros_dev"] = [
            jax.device_put(np.zeros((N_CORES * z.shape[0], *z.shape[1:]), z.dtype))
            for z in zero_outs
        ]
    out_arrs = sharded(*concat_in, *_CACHE["zeros_dev"])
    # outputs come back concatenated on axis 0 (N_CORES * dim0, ...)
    split = []
    for i, name in enumerate(out_names):
        arr = np.asarray(out_arrs[i])
        split.append(arr.reshape(N_CORES, arr.shape[0] // N_CORES, *arr.shape[1:]))
    return [
        {name: split[i][c] for i, name in enumerate(out_names)}
        for c in range(N_CORES)
    ]


def kernel(node_features, w1, b1, w2, b2):
    node_features = np.asarray(node_features, np.float32)
    w1 = np.asarray(w1, np.float32)
    b1 = np.asarray(b1, np.float32)
    w2 = np.asarray(w2, np.float32)
    b2 = np.asarray(b2, np.float32)
    in_maps = _host_prep(node_features, w1, b1, w2, b2)
    results = _run(in_maps)
    return _gather(results)

